# revision 32
# baseline (speedup 1.0000x reference)
"""Trainium2 Bass kernel for the MiniBatchAUC pairwise surrogate loss.

Math: with s = sigmoid(logits), pos/neg the 0/1 target masks,
    loss_sum = sum_{i in P, j in N} (1 - s_i + s_j)^2
factorizes exactly (expand the square; the double sum separates):
    loss_sum = n_neg * Sp2 + 2 * Sp1 * Sn1 + n_pos * Sn2
      Sp1 = sum_P (1-s),  Sp2 = sum_P (1-s)^2,
      Sn1 = sum_N s,      Sn2 = sum_N s^2,
and with c = sum T, m1 = sum T*s, m2 = sum T*s^2, g1 = sum s, g2 = sum s^2:
      Sp1 = c - m1, Sp2 = c - 2*m1 + m2, Sn1 = g1 - m1, Sn2 = g2 - m2.
So the O(N^2) pairwise matrix is never materialized: each core reduces its
2048-element shard to 5 per-partition partial sums; the host all-reduces
the per-core partials and applies the closed form.

Per-core device program (SPMD, identical on all 8 cores), raw bacc with
manual semaphores. The schedule is built around the fixed DMA latencies the
TRN2 cost model charges (HWDGE descriptor generation ~625ns, DGE->DMA
delay ~650ns, DMA-completion->semaphore propagation ~900ns):

  SP   : issues the in-DMA in the ENTRY block at t~0 (before the body
         branches); arrives at a barrier; issues the out-DMA once all five
         result columns retired; waits for its completion sem so the
         program cannot end before the output is globally visible.
  PE   : one EVSEM-range-clear of the kernel semaphores (device sem state
         persists across NEFF executions; stale values would let every
         wait_ge fall through and the program free-run on stale data -
         observed as intermittent corruption). It retires ~170ns into the
         run; the barrier holds every body wait until then, and the first
         semaphore ADD (the in-DMA completion) lands ~2μs later.
  ACT  : sigmoid (no accum_out - the accumulator read costs an extra 187ns
         engine slice and delays the semaphore every consumer waits on),
         then an independent second sigmoid whose fused accumulator yields
         g1 = sum(s) per partition, landing well before DVE's tail.
  DVE  : c = reduce(T) inside the sigmoid's ~460ns latency shadow, then
         three scalar_tensor_tensor ops whose fused accumulators yield m1,
         g2, m2 in one 77ns op each (STT's accum_out = sum(out) is the
         mul+reduce fusion; tensor_tensor_reduce would do the same but
         crashes this runtime, and attaching TWO sem updates to an
         accum-bearing STT breaks walrus codegen - one then_inc each).

The output path is a plain HWDGE DMA on SP. A SWDGE PREPARE_ONLY
dma_scatter_add + trigger_dma doorbell is ~1.2us faster in the model, but
couples correctness to the persistent on-device SWDGE ring FIFO: any run
killed between prep and trigger leaks an untriggered entry, after which
EVERY subsequent execution pops a stale descriptor set instead of its own
(observed: deterministic wrong results surviving process restarts, with
gpsimd.dma_reset making the replay worse, not better). The plain DMA keeps
no device state and leaves any stale ring entries dormant.

The entry all-engine barrier emitted by Bass.__init__ orders only the
const-AP memsets (Pool) against engine bodies; nothing here reads the const
APs, so it is stripped post-build and replaced by the program's own barrier
placed AFTER the sem-clear + in-DMA issue - the in-DMA then starts at t~0
instead of t~666, and the clear+barrier hide inside its ~2.3us completion
latency. The Block exit barrier (per-engine Drain + EVSEM handshake) is
also stripped: the only DMAs in flight are semaphore-quiesced before SP's
final wait, so engines may retire independently.

Inputs travel as bf16 (halves the in-DMA payload): logits lose ~3 decimal
digits, worth ~3e-6 relative on the final loss after the 16K-element sums;
targets are 0/1, exact in bf16.

"""

import numpy as np

try:
    import concourse.bass as bass
except ImportError:  # concourse ships in the container, not on sys.path
    import sys

    sys.path.insert(0, "/opt/trn_rl_repo")
    import concourse.bass as bass

from concourse import bacc, mybir
from concourse import bass_utils

N = 16384
NCORES = 8
SHARD = N // NCORES  # 2048 elements per core
P = 128  # SBUF partitions
F = SHARD // P  # 16 free elements per partition

f32 = mybir.dt.float32
bf16 = mybir.dt.bfloat16

_CACHE: dict = {}


def _strip_barriers(nc):
    """Remove the Drain+EventSemaphore all-engine barrier that Bass.__init__
    appends to the entry block (only the FIRST 11 such instructions - the
    program's own mid-entry barrier comes later in the list and must stay),
    and the Block exit barrier."""
    entry = nc.main_func.blocks[0]
    keep, removed = [], 0
    for inst in entry.instructions:
        if removed < 11 and isinstance(
            inst, (mybir.InstDrain, mybir.InstEventSemaphore)
        ):
            removed += 1
            continue
        keep.append(inst)
    assert removed == 11, f"expected 11 entry-barrier instructions, got {removed}"
    entry.instructions[:] = keep

    end = next(b for b in nc.main_func.blocks if b.name.endswith("_end"))
    end.instructions[:] = [
        inst
        for inst in end.instructions
        if not isinstance(inst, (mybir.InstDrain, mybir.InstEventSemaphore))
    ]


def _build():
    nc = bacc.Bacc(
        "TRN2",
        target_bir_lowering=False,
        debug=False,
        enable_asserts=False,
        num_devices=NCORES,
    )
    x_dram = nc.dram_tensor("x", [P, 2 * F], bf16, kind="ExternalInput").ap()
    o_dram = nc.dram_tensor("o", [P, 5], f32, kind="ExternalOutput").ap()

    Sig = mybir.ActivationFunctionType.Sigmoid
    X = mybir.AxisListType.X

    with (
        nc.sbuf_tensor([P, 2 * F], bf16) as x,
        nc.sbuf_tensor([P, F], f32) as s,
        nc.sbuf_tensor([P, F], f32) as sjunk,
        nc.sbuf_tensor([P, F], f32) as s2,
        nc.sbuf_tensor([P, F], f32) as ts,
        nc.sbuf_tensor([P, F], f32) as ts2,
        nc.sbuf_tensor([P, 5], f32) as r,  # g1|g2|c|m1|m2
        nc.semaphore() as dsem,  # in-DMA complete
        nc.semaphore() as osem,  # out-DMA complete
        nc.semaphore() as ssem,  # sigmoid retired
        nc.semaphore() as wsem,  # DVE intermediates (ts, s2) retired
        nc.semaphore() as pwsem,  # Pool intermediate (ts2) retired
        nc.semaphore() as vsem,  # result columns retired (5 total)
    ):
        L = x[:, 0:F]
        T = x[:, F : 2 * F]

        # --- Entry block: clear the kernel sem range on the otherwise-idle
        # PE engine (off SP's critical path; retires ~170ns, while the
        # in-DMA's first semaphore ADD is at ~2290ns and every wait_ge is
        # barrier-held - no concurrent access window), and launch the
        # in-DMA on SP at t~0.
        sems = [dsem, osem, ssem, wsem, pwsem, vsem]
        nums = sorted(h.num for h in sems)
        assert nums == list(range(nums[0], nums[0] + len(nums))), nums
        nc.tensor.sem_clear(range(nums[0], nums[-1] + 1))
        nc.sync.dma_start(x[:], x_dram).then_inc(dsem, 16)
        # Barrier: no engine may process a body wait until the sem clear is
        # done. SP arrives after the in-DMA issue (~700); ACT's table load
        # starts right after release, finishing before the data lands.
        nc.all_engine_barrier()

        with nc.Block() as block:

            @block.sync
            def _(sync):
                sync.wait_ge(vsem, 4)  # c, g1, g2, m2 retired
                sync.wait_ge(wsem, 1)  # m1 (the ts STT) retired
                sync.dma_start(o_dram, r[:]).then_inc(osem, 16)
                sync.wait_ge(osem, 16)  # out writes visible before end

            @block.scalar
            def _(scalar):
                scalar.wait_ge(dsem, 16)
                nc.scalar.activation(s[:], L, Sig).then_inc(ssem, 1)
                # g1 on the otherwise-idle ACT: an independent second sigmoid
                # (reads L, not s - no self-chain) whose fused accumulator
                # yields the per-partition sum; lands before DVE's tail.
                nc.scalar.activation(
                    sjunk[:], L, Sig, accum_out=r[:, 0:1]
                ).then_inc(vsem, 1)

            @block.vector
            def _(vector):
                Mult = mybir.AluOpType.mult
                vector.wait_ge(dsem, 16)
                # c: independent of s, fills the sigmoid latency shadow
                nc.vector.reduce_sum(r[:, 2:3], T, axis=X).then_inc(vsem, 1)
                vector.wait_ge(ssem, 1)
                # scalar_tensor_tensor fuses the multiply and the row-sum in
                # one 77ns DVE op (tensor_tensor_reduce would too but crashes
                # this runtime; STT is a different opcode and works):
                #   m1 = sum((T*1)*s), g2 = sum((s*1)*s), m2 = sum((ts*1)*ts)
                nc.vector.scalar_tensor_tensor(
                    ts[:], T, 1.0, s[:], Mult, Mult, accum_out=r[:, 3:4]
                ).then_inc(wsem, 1)
                nc.vector.scalar_tensor_tensor(
                    s2[:], s[:], 1.0, s[:], Mult, Mult, accum_out=r[:, 1:2]
                ).then_inc(vsem, 1)
                vector.wait_ge(wsem, 1)  # ts retired
                nc.vector.scalar_tensor_tensor(
                    ts2[:], ts[:], 1.0, ts[:], Mult, Mult, accum_out=r[:, 4:5]
                ).then_inc(vsem, 1)

    _strip_barriers(nc)
    nc.compile()
    return nc


def _get_nc():
    if "nc" not in _CACHE:
        _CACHE["nc"] = _build()
    return _CACHE["nc"]


def make_in_maps(logits: np.ndarray, targets: np.ndarray) -> list[dict]:
    import ml_dtypes

    # bf16 halves the in-DMA payload (the kernel is latency-bound, but the
    # transfer leg is still ~35ns); logits lose ~3 decimal digits, which
    # after summing 2048 near-random roundings costs ~1e-4 relative on the
    # final loss (gate is 2e-2). Targets are 0/1: exact in bf16.
    lb = np.ascontiguousarray(logits, dtype=np.float32).astype(ml_dtypes.bfloat16)
    tb = np.asarray(targets).astype(ml_dtypes.bfloat16)  # 0/1; lossless
    in_maps = []
    for k in range(NCORES):
        sl = slice(k * SHARD, (k + 1) * SHARD)
        xk = np.empty((P, 2 * F), ml_dtypes.bfloat16)
        xk[:, 0:F] = lb[sl].reshape(P, F)
        xk[:, F : 2 * F] = tb[sl].reshape(P, F)
        in_maps.append({"x": xk})
    return in_maps


def combine(outs: np.ndarray) -> np.ndarray:
    """All-reduce the [NCORES, P, 5] partials and apply the closed form."""
    tot = outs.astype(np.float64).sum(axis=(0, 1))
    g1, g2, c, m1, m2 = tot
    n_pos = c
    n_neg = float(N) - c
    sp1 = c - m1
    sp2 = c - 2.0 * m1 + m2
    sn1 = g1 - m1
    sn2 = g2 - m2
    loss = (n_neg * sp2 + 2.0 * sp1 * sn1 + n_pos * sn2) / (n_pos * n_neg)
    return np.array(loss, dtype=np.float32)


def kernel(logits: np.ndarray, targets: np.ndarray, **run_kwargs):
    nc = _get_nc()
    res = bass_utils.run_bass_kernel_spmd(
        nc, make_in_maps(logits, targets), core_ids=list(range(NCORES)), **run_kwargs
    )
    outs = np.stack([r["o"] for r in res.results])  # [8, 128, 5]
    out = combine(outs)
    _CACHE["last_results"] = res
    return out
